# revision 38
# baseline (speedup 1.0000x reference)
"""Trainium2 Bass kernel for nn_Attention_41996190220419.

Single-head causal attention with softplus weights and a time-flipped
rotary embedding, B=8 T=2048 C=1024 fp32.

Sharding: pure data-parallel over batch (1 batch element per NeuronCore,
8 cores, no collectives).

Numerics: V / OT / projection matmuls in bf16 (fp32 PSUM); the K/Q input
GEMMs and the K.Q^T score matmul run in fp8-e4m3 with DoubleRow perf mode
(2 contraction subtiles per PE pass -> ~1.8x per-matmul speedup).
Empirical L2 rel err of this mix vs the fp32 reference: ~1.0e-2 (gate:
2e-2).  fp8 scales: x*16, Wk/Wq*1024, rotated k/q *32 (all well inside
the +-240 TRN-e4m3 range); descaled in the ACT ops that read the PSUMs.

Inputs prepared on the host per core: x cast bf16; x8 = fp8(16*x^T) in
DoubleRow paired layout (streamed per chunk, no on-chip transpose or
quantization for the fp8 path); xt = x^T for the bf16 V GEMM via the
XBAR DMA-transpose (no PE transposes at all); wk8/wq8 pre-paired fp8.

Schedule (per core), software-pipelined over the four 512-token chunks
(K/Q GEMMs for chunk ch+1 are emitted between OT and projection of
span ch, so each rotation chain has ~30us of PE work covering it):
  prologue: K/Q GEMMs chunk 0 (fp8 DR) + bias(ACT) + rotation (DVE)
            -> fp8 paired k8 (chunk-local ring) / q8 (resident, full T)
  loop ch:  V GEMM (bf16, xt via XBAR DMA) -> resident vsb
            scores span ch = fp8 DR matmuls (4 pair-groups),
              softplus = Ln(Exp(x)+1) on ACT, diagonal-block masks (DVE)
            OT = V^T @ S (bf16, psB)
            K/Q GEMMs + rotation for chunk ch+1
            projection (bf16, psP) + bias (DVE) -> DRAM
PSUM pools split by reader so bank-reuse never waits on the slow DVE
path: psA(4) K/Q+scores (ACT-read), psB(3) V+OT, psP(1) projection.

The even/odd rotation pairs are turned into tile-level structure by
permuting the columns of Wk/Wq (and bk/bq) on the host to [evens|odds];
scores are invariant to any channel permutation applied to both K and Q.
cos/sin tables and diagonal masks are precomputed on the host.

Timeline-sim estimate 265us/core; bf16 baseline was 399us sim / 421us
measured HW.  L2 rel err on HW: 1.006e-2 (gate 2e-2).
"""

import os
import sys

if "/opt/trn_rl_repo" not in sys.path:
    sys.path.insert(0, "/opt/trn_rl_repo")

import numpy as np
import ml_dtypes

import concourse.bass as bass
import concourse.bacc as bacc
import concourse.mybir as mybir
import concourse.tile as tile
from concourse.bass_utils import run_bass_kernel_spmd

B, T, C = 8, 2048, 1024
H = C // 2
NCORES = 8
PD = 128
TCH = 512                 # t-chunk width (phase 1) == i-span width (attention)
NT = T // PD              # 16
NSP = T // TCH            # 4
NG = C // PD              # 8
NP4 = NG // 2             # 4 fp8 DoubleRow pair-groups
BF16 = mybir.dt.bfloat16
F32 = mybir.dt.float32
FP8 = mybir.dt.float8e4
AF = mybir.ActivationFunctionType
DR = mybir.MatmulPerfMode.DoubleRow
INV_SQRT_C = float(C) ** -0.5

SX = 16.0                 # fp8 scale on x
SW = 1024.0               # fp8 scale on Wk/Wq
SR = 32.0                 # fp8 scale on rotated k/q

_CACHE = {}

LAST_RESULT = None  # BassKernelResults of the most recent run (for profiling)


def _patch_act_tables():
    """Force every ACT func we use (Copy/Identity/Exp/Ln) to resolve to the
    single `natural_log_exp_and_others` table so the Exp/Ln alternation in
    the softplus does not thrash ACT_TABLE_LOADs (1.3us each, ~80 of them).
    Table ids are positional, so keep the dict order and only strip
    functions from the other tables."""
    if _CACHE.get("act_patched"):
        return
    from concourse import hw_specs
    orig = hw_specs.get_activation_tables
    combined = "natural_log_exp_and_others"

    def patched(arch):
        tables = orig(arch)
        if combined in tables:
            keep = tables[combined]
            tables = {
                name: (s if name == combined else (s - keep))
                for name, s in tables.items()
            }
        return tables

    hw_specs.get_activation_tables = patched
    bacc.get_activation_tables = patched
    _CACHE["act_patched"] = True


def _build_nc():
    _patch_act_tables()
    nc = bacc.Bacc("TRN2", target_bir_lowering=False, debug=False,
                   num_devices=NCORES)

    x_d = nc.dram_tensor("x", [T, C], BF16, kind="ExternalInput").ap()
    x8_d = nc.dram_tensor("x8", [NP4, PD, 2, T], FP8,
                          kind="ExternalInput").ap()
    wk8_d = nc.dram_tensor("wk8", [NP4, PD, 2, C], FP8,
                           kind="ExternalInput").ap()
    wq8_d = nc.dram_tensor("wq8", [NP4, PD, 2, C], FP8,
                           kind="ExternalInput").ap()
    wv_d = nc.dram_tensor("wv", [C, C], BF16, kind="ExternalInput").ap()
    wp_d = nc.dram_tensor("wp", [C, C], BF16, kind="ExternalInput").ap()
    bkr_d = nc.dram_tensor("bkr", [PD, NG], F32, kind="ExternalInput").ap()
    bqr_d = nc.dram_tensor("bqr", [PD, NG], F32, kind="ExternalInput").ap()
    bvb_d = nc.dram_tensor("bvb", [PD, C], F32, kind="ExternalInput").ap()
    bpb_d = nc.dram_tensor("bpb", [PD, C], F32, kind="ExternalInput").ap()
    cos_d = nc.dram_tensor("cosT", [H, T], BF16, kind="ExternalInput").ap()
    sin_d = nc.dram_tensor("sinT", [H, T], BF16, kind="ExternalInput").ap()
    msk_d = nc.dram_tensor("masks", [NSP, PD, TCH], BF16,
                           kind="ExternalInput").ap()
    out_d = nc.dram_tensor("out", [T, C], F32, kind="ExternalOutput").ap()

    with tile.TileContext(nc) as tc:
        with tc.tile_pool(name="persist", bufs=1) as pp, \
             tc.tile_pool(name="p1", bufs=1) as p1, \
             tc.tile_pool(name="at", bufs=1) as at, \
             tc.tile_pool(name="psA", bufs=4, space="PSUM") as psA, \
             tc.tile_pool(name="psB", bufs=3, space="PSUM") as psB, \
             tc.tile_pool(name="psP", bufs=1, space="PSUM") as psP:

            # q8 fp8 pairs, resident full T: q8p[p][:, 0, :] = rotated even
            # group p, [:, 1, :] = rotated odd group p+4  (scale SR)
            q8p = [pp.tile([PD, 2, T], FP8, tag=f"q8_{p}", name=f"q8_{p}")
                   for p in range(NP4)]
            vsb = [pp.tile([PD, C], BF16, tag=f"v{j}", name=f"v{j}")
                   for j in range(NT)]

            # x8 chunk 0 + wk8/wq8 first in the DMA queues: the K/Q GEMMs of
            # chunk 0 are the first PE work and depend only on these.
            def load_x8(ch):
                csl = slice(ch * TCH, (ch + 1) * TCH)
                x8 = [p1.tile([PD, 2, TCH], FP8, tag="x8", bufs=8,
                              name=f"x8_{ch}_{p}")
                      for p in range(NP4)]
                for p in range(NP4):
                    nc.sync.dma_start(out=x8[p], in_=x8_d[p][:, :, csl])
                return x8

            bkr = pp.tile([PD, NG], F32, name="bkr")
            nc.sync.dma_start(out=bkr, in_=bkr_d)
            bqr = pp.tile([PD, NG], F32, name="bqr")
            nc.sync.dma_start(out=bqr, in_=bqr_d)
            wk8 = []
            wq8 = []
            wt = pp.tile([PD, 2, C], FP8, tag="wk80", name="wk80")
            nc.sync.dma_start(out=wt, in_=wk8_d[0])
            wk8.append(wt)
            x8_cur = load_x8(0)
            for p in range(1, NP4):
                wt = pp.tile([PD, 2, C], FP8, tag=f"wk8{p}", name=f"wk8{p}")
                nc.sync.dma_start(out=wt, in_=wk8_d[p])
                wk8.append(wt)
            for p in range(NP4):
                wt = pp.tile([PD, 2, C], FP8, tag=f"wq8{p}", name=f"wq8{p}")
                nc.sync.dma_start(out=wt, in_=wq8_d[p])
                wq8.append(wt)

            def load_trig(ch):
                csl = slice(ch * TCH, (ch + 1) * TCH)
                trig = {}
                for e in range(4):
                    cs = p1.tile([PD, TCH], BF16, tag="trig", bufs=16,
                                 name=f"cs{e}_{ch}")
                    nc.sync.dma_start(out=cs,
                                      in_=cos_d[e * PD:(e + 1) * PD, csl])
                    sn = p1.tile([PD, TCH], BF16, tag="trig", bufs=16,
                                 name=f"sn{e}_{ch}")
                    nc.sync.dma_start(out=sn,
                                      in_=sin_d[e * PD:(e + 1) * PD, csl])
                    trig[e] = (cs, sn)
                return trig

            trig_cur = load_trig(0)

            # the first V GEMM runs after the chunk-0 K/Q GEMMs, so x
            # transposes and wv follow the fp8 streams in the queue
            xt0 = p1.tile([PD, NG, TCH], BF16, tag="xt", bufs=2, name="xt0")
            nc.sync.dma_start_transpose(xt0, x_d[0:TCH, :])
            wvsb = []
            for ci in range(NG):
                wt = p1.tile([PD, C], BF16, tag="wv", bufs=NG, name=f"wv{ci}")
                nc.sync.dma_start(out=wt, in_=wv_d[ci * PD:(ci + 1) * PD, :])
                wvsb.append(wt)

            bvb = pp.tile([PD, C], F32, name="bvb")
            nc.sync.dma_start(out=bvb, in_=bvb_d)
            bpb = pp.tile([PD, C], F32, name="bpb")
            mskt = []
            for d in range(NSP):
                m = pp.tile([PD, TCH], BF16, tag=f"msk{d}", name=f"msk{d}")
                nc.sync.dma_start(out=m, in_=msk_d[d])
                mskt.append(m)
            wpsb = []

            def load_late_consts():
                nc.sync.dma_start(out=bpb, in_=bpb_d)
                for ci in range(NG):
                    wt = pp.tile([PD, C], BF16, tag=f"wp{ci}", name=f"wp{ci}")
                    nc.sync.dma_start(out=wt,
                                      in_=wp_d[ci * PD:(ci + 1) * PD, :])
                    wpsb.append(wt)

            def emit_kq(ch, x8, trig):
                """K/Q GEMMs (fp8 DR) + bias + rotation -> fp8 k8 (returned)
                and q8p[:, ch span]."""
                csl = slice(ch * TCH, (ch + 1) * TCH)
                k8 = [p1.tile([PD, 2, TCH], FP8, tag="k8", bufs=8,
                              name=f"k8_{ch}_{p}")
                      for p in range(NP4)]
                for wname, w8, brt in (("k", wk8, bkr), ("q", wq8, bqr)):
                    for e in range(4):
                        o = e + 4
                        tmp = {}
                        for g in (e, o):
                            ps = psA.tile([PD, TCH], F32, tag="ps_mm",
                                          name=f"p{wname}{g}_{ch}")
                            for p in range(NP4):
                                nc.tensor.matmul(
                                    ps,
                                    lhsT=w8[p][:, :, g * PD:(g + 1) * PD],
                                    rhs=x8[p],
                                    start=(p == 0), stop=(p == NP4 - 1),
                                    perf_mode=DR)
                            # psum = (SX*x)@(SW*W); want SR*(xW + bias)
                            kt = p1.tile([PD, TCH], BF16, tag="kttmp",
                                         bufs=6, name=f"kt{wname}{g}_{ch}")
                            nc.scalar.activation(kt, ps, AF.Identity,
                                                 bias=brt[:, g:g + 1],
                                                 scale=SR / (SX * SW))
                            tmp[g] = kt
                        if wname == "k":
                            d0 = k8[e][:, 0, :]
                            d1 = k8[e][:, 1, :]
                        else:
                            d0 = q8p[e][:, 0, csl]
                            d1 = q8p[e][:, 1, csl]
                        cs, sn = trig[e]
                        ze, zo = tmp[e], tmp[o]
                        t1 = p1.tile([PD, TCH], BF16, tag="rot", bufs=4,
                                     name=f"r1{wname}{e}_{ch}")
                        nc.vector.tensor_mul(t1, ze, cs)
                        t2 = p1.tile([PD, TCH], BF16, tag="rot", bufs=4,
                                     name=f"r2{wname}{e}_{ch}")
                        nc.vector.tensor_mul(t2, zo, sn)
                        nc.vector.tensor_add(d0, t1, t2)
                        t3 = p1.tile([PD, TCH], BF16, tag="rot", bufs=4,
                                     name=f"r3{wname}{e}_{ch}")
                        nc.vector.tensor_mul(t3, zo, cs)
                        t4 = p1.tile([PD, TCH], BF16, tag="rot", bufs=4,
                                     name=f"r4{wname}{e}_{ch}")
                        nc.vector.tensor_mul(t4, ze, sn)
                        nc.vector.tensor_sub(d1, t3, t4)
                return k8

            # chunk-0 K/Q emitted up front; thereafter chunk ch+1's K/Q GEMMs
            # run between V(ch) and attention(ch) so every rotation chain is
            # covered by ~30us of PE work before its scores consume it.
            k8_cur = emit_kq(0, x8_cur, trig_cur)

            for ch in range(NSP):
                csl = slice(ch * TCH, (ch + 1) * TCH)

                # xt: [128, NG, TCH] transposed x for this chunk (XBAR DMA)
                if ch == 0:
                    xt = xt0
                else:
                    xt = p1.tile([PD, NG, TCH], BF16, tag="xt", bufs=2,
                                 name=f"xt{ch}")
                    nc.sync.dma_start_transpose(xt, x_d[csl, :])

                # V GEMM for this chunk's 4 t-tiles (bf16)
                for tt in range(4):
                    jt = ch * 4 + tt
                    for h in range(2):
                        ps = psB.tile([PD, TCH], F32, tag="ps_b",
                                      name=f"pv{jt}_{h}")
                        for ci in range(NG):
                            nc.tensor.matmul(
                                ps,
                                lhsT=xt[:, ci, tt * PD:(tt + 1) * PD],
                                rhs=wvsb[ci][:, h * TCH:(h + 1) * TCH],
                                start=(ci == 0), stop=(ci == NG - 1))
                        nc.vector.tensor_add(vsb[jt][:, h * TCH:(h + 1) * TCH],
                                             ps, bvb[:, h * TCH:(h + 1) * TCH])

                # next chunk's K/Q GEMMs + rotation (pipelined ahead)
                k8 = k8_cur

                # ---------------- attention span ch ----------------------
                s = ch
                nj = 4 * (s + 1)
                stact = []
                for j in range(nj):
                    ps = psA.tile([PD, TCH], F32, tag="ps_mm",
                                  name=f"pst{s}_{j}")
                    for p in range(NP4):
                        nc.tensor.matmul(
                            ps,
                            lhsT=q8p[p][:, :, j * PD:(j + 1) * PD],
                            rhs=k8[p],
                            start=(p == 0), stop=(p == NP4 - 1),
                            perf_mode=DR)
                    # softplus(x) = ln(1 + exp(x)); scores/sqrt(C) are
                    # bounded to a few units so exp cannot overflow
                    se = at.tile([PD, TCH], F32, tag="stexp", bufs=3,
                                 name=f"se{s}_{j}")
                    nc.scalar.activation(se, ps, AF.Exp,
                                         scale=INV_SQRT_C / (SR * SR))
                    st = at.tile([PD, TCH], BF16, tag="stact", bufs=17,
                                 name=f"st{s}_{j}")
                    nc.scalar.activation(st, se, AF.Ln, bias=1.0)
                    d = j - 4 * s
                    if d >= 0:
                        nc.vector.tensor_mul(st, st, mskt[d])
                    stact.append(st)

                ot = []
                for g in range(NG):
                    ps2 = psB.tile([PD, TCH], F32, tag="ps_b",
                                   name=f"pot{s}_{g}")
                    for j in range(nj):
                        nc.tensor.matmul(
                            ps2,
                            lhsT=vsb[j][:, g * PD:(g + 1) * PD],
                            rhs=stact[j],
                            start=(j == 0), stop=(j == nj - 1))
                    o = at.tile([PD, TCH], BF16, tag="ot", bufs=10,
                                name=f"ot{s}_{g}")
                    nc.scalar.activation(o, ps2, AF.Copy)
                    ot.append(o)

                # next chunk's K/Q GEMMs + rotation, emitted between OT and
                # proj: their DVE rotations queue after this span's masks but
                # before the proj bias adds, and the proj matmuls cover them.
                if ch + 1 < NSP:
                    x8_nxt = load_x8(ch + 1)
                    trig_nxt = load_trig(ch + 1)
                    k8_cur = emit_kq(ch + 1, x8_nxt, trig_nxt)
                if ch == 0:
                    load_late_consts()

                for tt in range(4):
                    trow = s * TCH + tt * PD
                    for h in range(2):
                        ps = psP.tile([PD, TCH], F32, tag="ps_pr",
                                      name=f"ppr{s}_{tt}_{h}")
                        for g in range(NG):
                            nc.tensor.matmul(
                                ps,
                                lhsT=ot[g][:, tt * PD:(tt + 1) * PD],
                                rhs=wpsb[g][:, h * TCH:(h + 1) * TCH],
                                start=(g == 0), stop=(g == NG - 1))
                        ob = at.tile([PD, TCH], F32, tag="ob", bufs=4,
                                     name=f"ob{s}_{tt}_{h}")
                        nc.vector.tensor_add(ob, ps,
                                             bpb[:, h * TCH:(h + 1) * TCH])
                        nc.sync.dma_start(
                            out=out_d[trow:trow + PD, h * TCH:(h + 1) * TCH],
                            in_=ob)
    nc.finalize()
    return nc


def _static_tables():
    if "tables" in _CACHE:
        return _CACHE["tables"]
    perm = np.concatenate([np.arange(0, C, 2), np.arange(1, C, 2)])
    j = np.arange(H, dtype=np.float64)
    t = (T - 1 - np.arange(T)).astype(np.float64)
    ang = np.outer(j, t)                      # [H, T], angle of pair j at time t
    cosT = np.cos(ang).astype(ml_dtypes.bfloat16)
    sinT = np.sin(ang).astype(ml_dtypes.bfloat16)
    a = np.arange(PD)[:, None]
    b = np.arange(TCH)[None, :]
    masks = np.stack([(a + PD * d <= b) for d in range(NSP)])
    masks = masks.astype(ml_dtypes.bfloat16)
    ident = np.eye(PD, dtype=ml_dtypes.bfloat16)
    _CACHE["tables"] = (perm, cosT, sinT, masks, ident)
    return _CACHE["tables"]


def _pack_w8(W, perm):
    """[C,C] -> fp8 paired layout [NP4, PD, 2, C]: c_in = p*256+half*128+row."""
    w = (np.asarray(W, np.float64)[:, perm] * SW)
    w8 = w.astype(ml_dtypes.float8_e4m3)
    w8 = w8.reshape(NP4, 2, PD, C).transpose(0, 2, 1, 3)
    return np.ascontiguousarray(w8)


def prepare(x, Wk, bk, Wq, bq, Wv, bv, Wp, bp):
    """Build (cached) the Bass program and the per-core input maps."""
    x = np.asarray(x, dtype=np.float32)
    Wk, bk = np.asarray(Wk, np.float32), np.asarray(bk, np.float32)
    Wq, bq = np.asarray(Wq, np.float32), np.asarray(bq, np.float32)
    Wv, bv = np.asarray(Wv, np.float32), np.asarray(bv, np.float32)
    Wp, bp = np.asarray(Wp, np.float32), np.asarray(bp, np.float32)

    perm, cosT, sinT, masks, ident = _static_tables()

    wk8 = _pack_w8(Wk, perm)
    wq8 = _pack_w8(Wq, perm)
    wv = Wv.astype(ml_dtypes.bfloat16)
    wp = Wp.astype(ml_dtypes.bfloat16)
    bkr = np.ascontiguousarray(
        (SR * bk[perm].astype(np.float64)).reshape(NG, PD).T).astype(np.float32)
    bqr = np.ascontiguousarray(
        (SR * bq[perm].astype(np.float64)).reshape(NG, PD).T).astype(np.float32)
    bvb = np.ascontiguousarray(np.broadcast_to(bv, (PD, C))).astype(np.float32)
    bpb = np.ascontiguousarray(np.broadcast_to(bp, (PD, C))).astype(np.float32)

    if "nc" not in _CACHE:
        _CACHE["nc"] = _build_nc()
    nc = _CACHE["nc"]

    shared = dict(wk8=wk8, wq8=wq8, wv=wv, wp=wp, bkr=bkr, bqr=bqr,
                  bvb=bvb, bpb=bpb, cosT=cosT, sinT=sinT, masks=masks)
    xb = x.astype(ml_dtypes.bfloat16)
    in_maps = []
    for i in range(NCORES):
        xi = np.ascontiguousarray(xb[i])
        # x8[p, row, half, t] = SX * x[t, c],  c = p*256 + half*128 + row
        xT = (xi.astype(np.float32).T * SX).astype(ml_dtypes.float8_e4m3)
        x8 = np.ascontiguousarray(
            xT.reshape(NP4, 2, PD, T).transpose(0, 2, 1, 3))
        in_maps.append(dict(x=xi, x8=x8, **shared))
    return nc, in_maps


def kernel(x, Wk, bk, Wq, bq, Wv, bv, Wp, bp):
    global LAST_RESULT
    nc, in_maps = prepare(x, Wk, bk, Wq, bq, Wv, bv, Wp, bp)
    res = run_bass_kernel_spmd(nc, in_maps, list(range(NCORES)))
    LAST_RESULT = res
    out = np.stack([res.results[i]["out"] for i in range(NCORES)], axis=0)
    return out.astype(np.float32)
